# revision 2
# baseline (speedup 1.0000x reference)
"""SS2D (VMamba-style 2D selective scan) Trainium2 Bass kernel, v5.2.

Full inputs in, full output out. Internally: 8-core SPMD.

Core c = b*4 + dh*2 + nh  (b batch, dh d_inner half, nh d_state half).

Structure:
  - own-half conv+in-proj fused host-side (9 dense tap matrices on a padded
    image, tensor engine); other-half conv via a 9-tap per-partition-scalar
    chain on Vector over an in-projected padded image (splits the B-phase
    work across both engines).
  - scan phase all-Vector (GpSimd big ops slow concurrent Vector ops);
    collectives are the only thing on the Pool queue so they never block
    compute.
  - comm: per-direction pair ReduceScatters (k=1's hides under k=0 phase);
    LayerNorm stats cross-dh via quad AllGather + selector matmul.
  - post: out-proj matmuls on (y*z) and z run during the AllGather flight;
    only the row-broadcast rescale happens after it. LayerNorm gamma/beta
    are folded into the out-proj weights host-side.
  - host adds the two dh-half out-projection partials.
"""

import numpy as np
import ml_dtypes

import concourse.bacc as bacc
import concourse.tile as tile
import concourse.mybir as mybir
from concourse.bass_utils import run_bass_kernel_spmd

FP32 = mybir.dt.float32
BF16 = mybir.dt.bfloat16
AF = mybir.ActivationFunctionType
OP = mybir.AluOpType

B, DM, D, N, R, K = 2, 96, 192, 16, 6, 2
NCORES = 8

PAIRS = [[0, 1], [2, 3], [4, 5], [6, 7]]
QUADS = [[0, 1, 2, 3], [4, 5, 6, 7]]


def diag_order(size):
    locs = [i * size + i for i in range(size)]
    for i in range(size):
        for j in range(i + 1, size):
            locs.append(i * size + j)
    for i in range(size):
        for j in range(i):
            locs.append(i * size + j)
    return np.asarray(locs, dtype=np.int64)


def diag_segments(H):
    segs = [(0, 0, H, H + 1)]
    p = H
    for i in range(H):
        ln = H - 1 - i
        if ln > 0:
            segs.append((p, i * H + i + 1, ln, 1))
            p += ln
    for i in range(H):
        ln = i
        if ln > 0:
            segs.append((p, i * H, ln, 1))
            p += ln
    assert p == H * H
    return segs


_PROGRAMS = {}


def build_program(H=64, has_beta=False):
    W = H
    L = H * W
    LH = L // 2
    FC = 512
    NF = L // FC
    RPC = FC // W
    PADL = (H + 2) * (W + 2)
    segs = diag_segments(H)

    nc = bacc.Bacc("TRN2", target_bir_lowering=False, debug=False,
                   enable_asserts=False, num_devices=NCORES)

    def din(name, shape, dt=BF16):
        return nc.dram_tensor(name, shape, dt, kind="ExternalInput").ap()

    i_xpad = din("xpad", [96, PADL])
    i_xTh = din("xTh", [96, LH])
    i_wfu = din("wfu", [96, 2 * 9 * 96])
    i_cb = din("cb", [96, 2], FP32)
    i_xpj = din("xpj", [96, 2 * 44])
    i_dtw = din("dtw", [12, 96])
    i_dtb = din("dtb", [96, 2], FP32)
    i_A = din("Acol", [96, 16], FP32)
    i_dse = din("dse", [96, 1], FP32)
    i_id96 = din("id96", [96, 96])
    i_onc = din("onc", [96, 1])
    i_onr = din("onr", [1, 96])
    i_msel = din("msel", [8, 2])
    i_wz = din("wz", [96, 96])
    i_wo = din("wo", [96, 96])
    i_wob = din("wob", [96, 96])
    o_out = nc.dram_tensor("out_part", [96, LH], FP32,
                           kind="ExternalOutput").ap()

    with tile.TileContext(nc) as tc:
        with tc.tile_pool(name="cst", bufs=1) as cst, \
             tc.tile_pool(name="big", bufs=1) as big, \
             tc.tile_pool(name="tmp", bufs=3) as tmp, \
             tc.tile_pool(name="nlp", bufs=2) as nlp, \
             tc.tile_pool(name="pst", bufs=1) as pst, \
             tc.tile_pool(name="ps", bufs=1, space="PSUM") as ps, \
             tc.tile_pool(name="drm", bufs=1, space="DRAM") as drm:

            ps_ctr = [0]

            def ps_tile(shape, tag=None):
                t = ps.tile(shape, FP32, tag=tag or f"y{ps_ctr[0] % 8}")
                if tag is None:
                    ps_ctr[0] += 1
                return t

            ld_ctr = [0]

            def load(ap_in, shape, dt=BF16, pool=cst, nm=None):
                nm = nm or f"c_{ap_in.tensor.name}"
                t = pool.tile(shape, dt, tag=nm, name=nm)
                eng = nc.sync if ld_ctr[0] % 2 == 0 else nc.scalar
                ld_ctr[0] += 1
                eng.dma_start(out=t, in_=ap_in)
                return t

            # ---- constants (alternating DMA queues)
            t_xpad = load(i_xpad, [96, H + 2, W + 2])
            t_wfu = load(i_wfu, [96, 2, 9, 96])
            t_cb = load(i_cb, [96, 2], FP32)
            t_xpj = load(i_xpj, [96, 2, 44])
            t_dtw0 = load(i_dtw[0:6, :], [6, 96], nm='c_dtw0')
            t_dtw1 = load(i_dtw[6:12, :], [6, 96], nm='c_dtw1')
            t_dtws = [t_dtw0, t_dtw1]
            t_dtb = load(i_dtb, [96, 2], FP32)
            t_A = load(i_A, [96, 16], FP32)
            t_dse = load(i_dse, [96, 1], FP32)
            t_id96 = load(i_id96, [96, 96])
            t_onc = load(i_onc, [96, 1])
            t_onr = load(i_onr, [1, 96])
            t_msel = load(i_msel, [8, 2])
            t_wz = load(i_wz, [96, 96])
            t_wo = load(i_wo, [96, 96])
            t_wob = load(i_wob, [96, 96])
            t_xTh = load(i_xTh, [96, LH])
            t_eps = cst.tile([1, 1], FP32)
            nc.vector.memset(t_eps, 1e-5)

            d_xd = drm.tile([44, L], BF16)
            d_s1 = drm.tile([192, LH], BF16)
            d_r1 = drm.tile([96, LH], BF16)
            d_s2a = drm.tile([192, LH // 2], BF16)
            d_s2b = drm.tile([192, LH // 2], BF16)
            d_r2a = drm.tile([96, LH // 2], BF16)
            d_r2b = drm.tile([96, LH // 2], BF16)
            d_st_in = drm.tile([2, LH], FP32)
            d_st_out = drm.tile([8, LH], FP32)

            # ---- phase B: fused conv+in-proj on tensor + SiLU, both halves;
            # diag reorder right after each half (big segments via DMA).
            t_xs0s = [None, None]
            for h in (0, 1):
                t_xsr = tmp.tile([96, L], BF16, tag="tmp16", name=f"t_xsr{h}")
                for f in range(NF):
                    p = ps_tile([96, RPC, W])
                    for t in range(9):
                        ky, kx = t // 3, t % 3
                        rhs = t_xpad[:, f * RPC + ky:f * RPC + ky + RPC,
                                     kx:kx + W]
                        nc.tensor.matmul(p, t_wfu[:, h, t, :], rhs,
                                         start=(t == 0), stop=(t == 8))
                    p2 = p.rearrange("p r w -> p (r w)")
                    fs = slice(f * FC, (f + 1) * FC)
                    nc.scalar.activation(out=t_xsr[:, fs], in_=p2,
                                         func=AF.Silu,
                                         bias=t_cb[:, h:h + 1], scale=1.0)
                t_xs0 = big.tile([96, L], BF16, tag=f"xs0_{h}",
                                 name=f"t_xs0_{h}")
                dctr = 0
                for si, (dp, rp, ln, st) in enumerate(segs):
                    sg = t_xsr[:, rp:rp + (ln - 1) * st + 1:st] if st > 1 \
                        else t_xsr[:, rp:rp + ln]
                    if (si * 2 + h) % 4 < 3:
                        nc.vector.tensor_copy(out=t_xs0[:, dp:dp + ln], in_=sg)
                    else:
                        nc.gpsimd.tensor_copy(out=t_xs0[:, dp:dp + ln], in_=sg)
                t_xs0s[h] = t_xs0
            t_xs0 = t_xs0s[0]

            # ---- z-path: z = silu(Wz x), own half, diag order
            t_z = pst.tile([96, LH], BF16, tag="z")
            for f in range(LH // FC):
                fs = slice(f * FC, (f + 1) * FC)
                p = ps_tile([96, FC])
                nc.tensor.matmul(p, t_wz, t_xTh[:, fs], start=True, stop=True)
                nc.scalar.activation(out=t_z[:, fs], in_=p, func=AF.Silu,
                                     bias=0.0, scale=1.0)

            # ---- phase E: x_proj rows -> DRAM, dt expansion (both k)
            t_sgf = {}
            t_du = {}
            for k in (1, 0):
                t_sgf[k] = tmp.tile([96, L], BF16, tag="tmp16",
                                    name=f"t_sgf{k}")
            for f in range(NF):
                fs = slice(f * FC, (f + 1) * FC)
                for k in (1, 0):
                    p = ps_tile([22, FC])
                    for h in (0, 1):
                        nc.tensor.matmul(p, t_xpj[:, h, k * 22:(k + 1) * 22],
                                         t_xs0s[h][:, fs],
                                         start=(h == 0), stop=(h == 1))
                    t_xdc = nlp.tile([22, FC], BF16, tag="xdc", bufs=2)
                    if (f + k) % 2 == 0:
                        nc.vector.tensor_copy(out=t_xdc, in_=p)
                    else:
                        nc.scalar.copy(out=t_xdc, in_=p)
                    nc.sync.dma_start(out=d_xd[k * 22:k * 22 + 22, fs],
                                      in_=t_xdc)
                    p2 = ps_tile([96, FC])
                    nc.tensor.matmul(p2, t_dtws[k], t_xdc[0:6, :],
                                     start=True, stop=True)
                    nc.scalar.activation(out=t_sgf[k][:, fs],
                                         in_=p2, func=AF.Sigmoid,
                                         bias=t_dtb[:, k:k + 1], scale=-1.0)
            nc.scalar.activation(out=t_sgf[1], in_=t_sgf[1], func=AF.Ln,
                                 bias=0.0, scale=1.0)

            # ---- scan loop: k=1 first, then k=0; pair-RS after each
            while ps_ctr[0] % 8 != 0:
                ps_ctr[0] += 1

            for k in (1, 0):
                if k == 0:
                    nc.scalar.activation(out=t_sgf[0], in_=t_sgf[0],
                                         func=AF.Ln, bias=0.0, scale=1.0)
                t_d = big.tile([96, L], BF16, tag="du", name=f"t_du{k}")
                nc.vector.tensor_tensor(out=t_d, in0=t_sgf[k], in1=t_xs0,
                                        op=OP.mult)
                t_du[k] = t_d
                yps = [ps.tile([96, FC], FP32, tag=f"y{f}", name=f"yps{k}{f}")
                       for f in range(NF)]
                for j in range(8):
                    row_b = k * 22 + 6 + j
                    row_c = k * 22 + 14 + j
                    t_br = nlp.tile([96, L], BF16, tag="brep", bufs=2)
                    t_cr = nlp.tile([96, L], BF16, tag="crep", bufs=2)
                    nc.sync.dma_start(
                        out=t_br,
                        in_=d_xd[row_b:row_b + 1, :].to_broadcast((96, L)))
                    nc.scalar.dma_start(
                        out=t_cr,
                        in_=d_xd[row_c:row_c + 1, :].to_broadcast((96, L)))
                    t_dA = nlp.tile([96, L], BF16, tag="dA", bufs=2)
                    nc.scalar.activation(out=t_dA, in_=t_sgf[k], func=AF.Exp,
                                         bias=0.0,
                                         scale=t_A[:, k * 8 + j:k * 8 + j + 1])
                    nc.vector.tensor_tensor(out=t_br, in0=t_du[k],
                                            in1=t_br, op=OP.mult)
                    t_h = nlp.tile([96, L], BF16, tag="h", bufs=2)
                    if k == 0:
                        nc.vector.tensor_tensor_scan(
                            out=t_h, data0=t_dA, data1=t_br, initial=0.0,
                            op0=OP.mult, op1=OP.add)
                    else:
                        nc.vector.tensor_tensor_scan(
                            out=t_h[:, ::-1], data0=t_dA[:, ::-1],
                            data1=t_br[:, ::-1], initial=0.0,
                            op0=OP.mult, op1=OP.add)
                    nc.vector.tensor_tensor(out=t_h, in0=t_h, in1=t_cr,
                                            op=OP.mult)
                    for f in range(NF):
                        nc.tensor.matmul(yps[f], t_id96,
                                         t_h[:, f * FC:(f + 1) * FC],
                                         start=(j == 0), stop=(j == 7))
                # evict into the pair-RS input layout
                if k == 1:
                    forder = list(range(NF))
                    for i, f in enumerate(forder):
                        ro = (f // 4) * 96
                        cs = slice((f % 4) * FC, (f % 4) * FC + FC)
                        t_ev = nlp.tile([96, FC], BF16, tag="yev", bufs=2)
                        nc.scalar.activation(out=t_ev, in_=yps[f],
                                             func=AF.Identity, bias=0.0,
                                             scale=1.0)
                        eng = nc.sync if i % 2 == 0 else nc.scalar
                        eng.dma_start(out=d_s1[ro:ro + 96, cs], in_=t_ev)
                    nc.gpsimd.collective_compute(
                        "ReduceScatter", OP.add, replica_groups=PAIRS,
                        ins=[d_s1.opt()], outs=[d_r1.opt()])
                else:
                    # halves: a = f in (0,1,4,5) -> cols 0:1024 of own half
                    for gi, grp in enumerate(((0, 1, 4, 5), (2, 3, 6, 7))):
                        dst = d_s2a if gi == 0 else d_s2b
                        for i, f in enumerate(grp):
                            ro = (f // 4) * 96
                            cs = slice((f % 2) * FC, (f % 2) * FC + FC)
                            t_ev = nlp.tile([96, FC], BF16, tag="yev", bufs=2)
                            nc.vector.scalar_tensor_tensor(
                                out=t_ev, in0=t_xs0[:, f * FC:(f + 1) * FC],
                                scalar=t_dse, in1=yps[f], op0=OP.mult,
                                op1=OP.add)
                            eng = nc.sync if i % 2 == 0 else nc.scalar
                            eng.dma_start(out=dst[ro:ro + 96, cs], in_=t_ev)
                        nc.gpsimd.collective_compute(
                            "ReduceScatter", OP.add, replica_groups=PAIRS,
                            ins=[dst.opt()],
                            outs=[(d_r2a if gi == 0 else d_r2b).opt()])

            # ---- Q = Wo_g @ z (stats- and y-independent; fills RS#2 wait)
            t_q = pst.tile([96, LH], BF16, tag="q")
            for f in range(LH // FC):
                fs = slice(f * FC, (f + 1) * FC)
                p_qq = ps_tile([96, FC])
                nc.tensor.matmul(p_qq, t_wo, t_z[:, fs], start=True,
                                 stop=True)
                nc.vector.tensor_copy(out=t_q[:, fs], in_=p_qq)

            # ---- post: y = r1 + r2 (own-dh rows, own L-half), split halves
            t_ra = nlp.tile([96, LH], BF16, tag="rca", bufs=1)
            nc.sync.dma_start(out=t_ra, in_=d_r1)
            t_rb = nlp.tile([96, LH], BF16, tag="rcb", bufs=1)
            nc.sync.dma_start(out=t_rb[:, 0:LH // 2], in_=d_r2a)
            nc.scalar.dma_start(out=t_rb[:, LH // 2:LH], in_=d_r2b)
            t_y = t_ra
            nc.vector.tensor_tensor(out=t_y[:, 0:LH // 2],
                                    in0=t_ra[:, 0:LH // 2],
                                    in1=t_rb[:, 0:LH // 2], op=OP.add)
            nc.vector.tensor_tensor(out=t_y[:, LH // 2:LH],
                                    in0=t_ra[:, LH // 2:LH],
                                    in1=t_rb[:, LH // 2:LH], op=OP.add)

            # LN stats: own-dh partial sums / sumsq -> AllGather quad
            for f in range(LH // FC):
                fs = slice(f * FC, (f + 1) * FC)
                t_ysq = nlp.tile([96, FC], BF16, tag="ysq", bufs=2)
                nc.scalar.activation(out=t_ysq, in_=t_y[:, fs],
                                     func=AF.Square, bias=0.0, scale=1.0)
                p_s = ps_tile([1, FC])
                nc.tensor.matmul(p_s, t_onc, t_y[:, fs], start=True,
                                 stop=True)
                p_q = ps_tile([1, FC])
                nc.tensor.matmul(p_q, t_onc, t_ysq, start=True,
                                 stop=True)
                t_stc = nlp.tile([1, FC], FP32, tag="stc", bufs=2)
                nc.vector.tensor_copy(out=t_stc, in_=p_s)
                t_stq = nlp.tile([1, FC], FP32, tag="stq", bufs=2)
                nc.vector.tensor_copy(out=t_stq, in_=p_q)
                nc.sync.dma_start(out=d_st_in[0:1, fs], in_=t_stc)
                nc.sync.dma_start(out=d_st_in[1:2, fs], in_=t_stq)
            nc.gpsimd.collective_compute(
                "AllGather", OP.bypass, replica_groups=QUADS,
                ins=[d_st_in.opt()], outs=[d_st_out.opt()])

            # During AG flight: P1 = Wo_g @ (y*z); copies land in SBUF.
            t_p1 = pst.tile([96, LH], BF16, tag="p1")
            for f in range(LH // FC):
                fs = slice(f * FC, (f + 1) * FC)
                t_yz = nlp.tile([96, FC], BF16, tag="ysq", bufs=2)
                nc.vector.tensor_tensor(out=t_yz, in0=t_y[:, fs],
                                        in1=t_z[:, fs], op=OP.mult)
                p_p1 = ps_tile([96, FC])
                nc.tensor.matmul(p_p1, t_wo, t_yz, start=True, stop=True)
                nc.scalar.copy(out=t_p1[:, fs], in_=p_p1)

            # after AG: selector-matmul stats, mu/rstd rows, rescale rows
            for f in range(LH // FC):
                fs = slice(f * FC, (f + 1) * FC)
                t_staf = nlp.tile([8, FC], FP32, tag="stalf", bufs=2)
                nc.sync.dma_start(out=t_staf, in_=d_st_out[:, fs])
                t_sta = nlp.tile([8, FC], BF16, tag="stall", bufs=2)
                nc.vector.tensor_copy(out=t_sta, in_=t_staf)
                p_m = ps_tile([1, FC])
                nc.tensor.matmul(p_m, t_msel[:, 0:1], t_sta, start=True,
                                 stop=True)
                p_m2 = ps_tile([1, FC])
                nc.tensor.matmul(p_m2, t_msel[:, 1:2], t_sta, start=True,
                                 stop=True)
                t_mub = nlp.tile([1, FC], BF16, tag="mub", bufs=2)
                nc.vector.tensor_scalar_mul(out=t_mub, in0=p_m,
                                            scalar1=1.0 / D)
                t_m2 = nlp.tile([1, FC], FP32, tag="m2", bufs=2)
                nc.vector.tensor_scalar_mul(out=t_m2, in0=p_m2,
                                            scalar1=1.0 / D)
                t_mu2 = nlp.tile([1, FC], FP32, tag="mu2", bufs=2)
                nc.vector.tensor_tensor(out=t_mu2, in0=t_mub, in1=t_mub,
                                        op=OP.mult)
                t_var = nlp.tile([1, FC], FP32, tag="var", bufs=2)
                nc.vector.tensor_tensor(out=t_var, in0=t_m2, in1=t_mu2,
                                        op=OP.subtract)
                t_vre = nlp.tile([1, FC], FP32, tag="vre", bufs=2)
                nc.vector.tensor_scalar_add(out=t_vre, in0=t_var,
                                            scalar1=1e-5)
                t_iva = nlp.tile([1, FC], FP32, tag="iva", bufs=2)
                nc.vector.reciprocal_approx_fast(out=t_iva, in_=t_vre)
                t_rsb = nlp.tile([1, FC], BF16, tag="rsb", bufs=2)
                nc.scalar.activation(out=t_rsb, in_=t_iva, func=AF.Sqrt,
                                     bias=0.0, scale=1.0)

                p_bm = ps_tile([96, FC])
                nc.tensor.matmul(p_bm, t_onr, t_mub, start=True, stop=True)
                p_br = ps_tile([96, FC])
                nc.tensor.matmul(p_br, t_onr, t_rsb, start=True, stop=True)
                t_c1 = nlp.tile([96, FC], BF16, tag="c1", bufs=2)
                nc.vector.tensor_tensor(out=t_c1, in0=t_q[:, fs], in1=p_bm,
                                        op=OP.mult)
                t_c2 = nlp.tile([96, FC], BF16, tag="c2", bufs=2)
                nc.vector.tensor_tensor(out=t_c2, in0=t_p1[:, fs], in1=t_c1,
                                        op=OP.subtract)
                t_oc = nlp.tile([96, FC], FP32, tag="oc", bufs=2)
                if has_beta:
                    t_c3 = nlp.tile([96, FC], BF16, tag="c3", bufs=2)
                    nc.vector.tensor_tensor(out=t_c3, in0=t_c2, in1=p_br,
                                            op=OP.mult)
                    p_b3 = ps_tile([96, FC])
                    nc.tensor.matmul(p_b3, t_wob, t_z[:, fs], start=True,
                                     stop=True)
                    nc.vector.tensor_tensor(out=t_oc, in0=t_c3, in1=p_b3,
                                            op=OP.add)
                else:
                    nc.vector.tensor_tensor(out=t_oc, in0=t_c2, in1=p_br,
                                            op=OP.mult)
                nc.sync.dma_start(out=o_out[:, fs], in_=t_oc)

    nc.finalize()
    return nc


def get_program(H=64, has_beta=False):
    key = (H, has_beta)
    if key not in _PROGRAMS:
        _PROGRAMS[key] = build_program(H, has_beta)
    return _PROGRAMS[key]


# ---------------------------------------------------------------- host side

def make_in_maps(inputs, H=64):
    W = H
    L = H * W
    LH = L // 2
    bf = ml_dtypes.bfloat16
    order = diag_order(H)

    x = np.asarray(inputs["x"], np.float32)
    w_in = np.asarray(inputs["w_in"], np.float32)
    conv_w = np.asarray(inputs["conv_w"], np.float32)
    conv_b = np.asarray(inputs["conv_b"], np.float32)
    x_proj_w = np.asarray(inputs["x_proj_w"], np.float32)
    dt_w = np.asarray(inputs["dt_w"], np.float32)
    dt_b = np.asarray(inputs["dt_b"], np.float32)
    A_logs = np.asarray(inputs["A_logs"], np.float32)
    Ds = np.asarray(inputs["Ds"], np.float32)
    ln_g = np.asarray(inputs["ln_g"], np.float32)
    ln_b = np.asarray(inputs["ln_b"], np.float32)
    w_out = np.asarray(inputs["w_out"], np.float32)

    A_full = np.exp(A_logs).reshape(K, D, N)
    Ds2 = Ds.reshape(K, D)
    id96 = np.eye(96, dtype=np.float32)
    onc = np.ones((96, 1), np.float32)
    onr = np.ones((1, 96), np.float32)

    in_maps = []
    for c in range(NCORES):
        b, dh, nh = c // 4, (c // 2) % 2, c % 2
        dsl = slice(dh * 96, dh * 96 + 96)
        osl = slice((1 - dh) * 96, (1 - dh) * 96 + 96)

        xpad = np.zeros((96, H + 2, W + 2), np.float32)
        xpad[:, 1:H + 1, 1:W + 1] = np.transpose(x[b], (2, 0, 1))
        xT = x[b].reshape(L, DM).T
        xTh = np.ascontiguousarray(xT[:, order[nh * LH:(nh + 1) * LH]])

        # conv fused with in-proj (tensor path), both halves
        wfu = np.zeros((96, 2, 9, 96), np.float32)
        for hh, hsl in enumerate((dsl, osl)):
            cw = conv_w[hsl, 0].reshape(96, 9)
            for t in range(9):
                wfu[:, hh, t, :] = (w_in[hsl, :] * cw[:, t:t + 1]).T

        cb2 = np.stack([conv_b[dsl], conv_b[osl]], axis=1)
        xpj = np.zeros((96, 2, 44), np.float32)
        for hh, hsl in enumerate((dsl, osl)):
            for k in range(K):
                sel = np.concatenate([
                    x_proj_w[k, 0:R, hsl],
                    -x_proj_w[k, R + nh * 8:R + nh * 8 + 8, hsl],
                    x_proj_w[k, R + N + nh * 8:R + N + nh * 8 + 8, hsl],
                ], axis=0)
                xpj[:, hh, k * 22:(k + 1) * 22] = sel.T

        dtw = np.zeros((12, 96), np.float32)
        for k in range(K):
            dtw[k * 6:(k + 1) * 6, :] = dt_w[k, dsl, :].T

        dtb = -np.stack([dt_b[0, dsl], dt_b[1, dsl]], axis=1)
        Acol = np.concatenate([A_full[0, dsl, nh * 8:nh * 8 + 8],
                               A_full[1, dsl, nh * 8:nh * 8 + 8]], axis=1)
        dse = ((Ds2[0, dsl] + Ds2[1, dsl]) / 2.0)[:, None]
        msel = np.zeros((8, 2), np.float32)
        for rr in (dh * 2 + nh, (1 - dh) * 2 + nh):
            msel[rr * 2, 0] = 1.0
            msel[rr * 2 + 1, 1] = 1.0
        wz = np.ascontiguousarray(w_in[D + dh * 96:D + dh * 96 + 96, :].T)
        wo = np.ascontiguousarray(w_out[:, dsl].T * ln_g[dsl][:, None])
        wob = np.ascontiguousarray(w_out[:, dsl].T * ln_b[dsl][:, None])

        in_maps.append({
            "xpad": xpad.reshape(96, -1).astype(bf),
            "xTh": xTh.astype(bf),
            "wfu": wfu.reshape(96, -1).astype(bf),
            "cb": cb2.astype(np.float32),
            "xpj": xpj.reshape(96, -1).astype(bf),
            "dtw": dtw.astype(bf),
            "dtb": dtb.astype(np.float32),
            "Acol": Acol.astype(np.float32),
            "dse": dse.astype(np.float32),
            "id96": id96.astype(bf),
            "onc": onc.astype(bf),
            "onr": onr.astype(bf),
            "msel": msel.astype(bf),
            "wz": wz.astype(bf),
            "wo": wo.astype(bf),
            "wob": wob.astype(bf),
        })
    return in_maps


def assemble_output(results, H=64):
    L = H * H
    LH = L // 2
    order = diag_order(H)
    out = np.zeros((B, L, DM), np.float32)
    for b in range(B):
        for nh in range(2):
            acc = (results[b * 4 + nh]["out_part"] +
                   results[b * 4 + 2 + nh]["out_part"])
            out[b, order[nh * LH:(nh + 1) * LH], :] = acc.T
    return out.reshape(B, H, H, DM)


def kernel(**inputs):
    has_beta = bool(np.any(np.asarray(inputs["ln_b"]) != 0.0))
    nc = get_program(64, has_beta)
    in_maps = make_in_maps(inputs, 64)
    res = run_bass_kernel_spmd(nc, in_maps, core_ids=list(range(NCORES)))
    return assemble_output(res.results, 64)


# revision 3
# speedup vs baseline: 1.0009x; 1.0009x over previous
"""SS2D (VMamba-style 2D selective scan) Trainium2 Bass kernel, v5.2.

Full inputs in, full output out. Internally: 8-core SPMD.

Core c = b*4 + dh*2 + nh  (b batch, dh d_inner half, nh d_state half).

Structure:
  - own-half conv+in-proj fused host-side (9 dense tap matrices on a padded
    image, tensor engine); other-half conv via a 9-tap per-partition-scalar
    chain on Vector over an in-projected padded image (splits the B-phase
    work across both engines).
  - scan phase all-Vector (GpSimd big ops slow concurrent Vector ops);
    collectives are the only thing on the Pool queue so they never block
    compute.
  - comm: per-direction pair ReduceScatters (k=1's hides under k=0 phase);
    LayerNorm stats cross-dh via quad AllGather + selector matmul.
  - post: out-proj matmuls on (y*z) and z run during the AllGather flight;
    only the row-broadcast rescale happens after it. LayerNorm gamma/beta
    are folded into the out-proj weights host-side.
  - host adds the two dh-half out-projection partials.
"""

import numpy as np
import ml_dtypes

import concourse.bacc as bacc
import concourse.tile as tile
import concourse.mybir as mybir
from concourse.bass_utils import run_bass_kernel_spmd

FP32 = mybir.dt.float32
BF16 = mybir.dt.bfloat16
AF = mybir.ActivationFunctionType
OP = mybir.AluOpType

B, DM, D, N, R, K = 2, 96, 192, 16, 6, 2
NCORES = 8

PAIRS = [[0, 1], [2, 3], [4, 5], [6, 7]]
QUADS = [[0, 1, 2, 3], [4, 5, 6, 7]]


def diag_order(size):
    locs = [i * size + i for i in range(size)]
    for i in range(size):
        for j in range(i + 1, size):
            locs.append(i * size + j)
    for i in range(size):
        for j in range(i):
            locs.append(i * size + j)
    return np.asarray(locs, dtype=np.int64)


def diag_segments(H):
    segs = [(0, 0, H, H + 1)]
    p = H
    for i in range(H):
        ln = H - 1 - i
        if ln > 0:
            segs.append((p, i * H + i + 1, ln, 1))
            p += ln
    for i in range(H):
        ln = i
        if ln > 0:
            segs.append((p, i * H, ln, 1))
            p += ln
    assert p == H * H
    return segs


_PROGRAMS = {}


def build_program(H=64, has_beta=False):
    W = H
    L = H * W
    LH = L // 2
    FC = 512
    NF = L // FC
    RPC = FC // W
    PADL = (H + 2) * (W + 2)
    segs = diag_segments(H)

    nc = bacc.Bacc("TRN2", target_bir_lowering=False, debug=False,
                   enable_asserts=False, num_devices=NCORES)

    def din(name, shape, dt=BF16):
        return nc.dram_tensor(name, shape, dt, kind="ExternalInput").ap()

    i_xpad = din("xpad", [96, PADL])
    i_xTh = din("xTh", [96, LH])
    i_wfu = din("wfu", [96, 2 * 9 * 96])
    i_cb = din("cb", [96, 2], FP32)
    i_xpj = din("xpj", [96, 2 * 44])
    i_dtw = din("dtw", [12, 96])
    i_dtb = din("dtb", [96, 2], FP32)
    i_A = din("Acol", [96, 16], FP32)
    i_dse = din("dse", [96, 1], FP32)
    i_id96 = din("id96", [96, 96])
    i_onc = din("onc", [96, 1])
    i_onr = din("onr", [1, 96])
    i_msel = din("msel", [8, 2])
    i_wz = din("wz", [96, 96])
    i_wo = din("wo", [96, 96])
    i_wob = din("wob", [96, 96])
    o_out = nc.dram_tensor("out_part", [96, LH], FP32,
                           kind="ExternalOutput").ap()

    with tile.TileContext(nc) as tc:
        with tc.tile_pool(name="cst", bufs=1) as cst, \
             tc.tile_pool(name="big", bufs=1) as big, \
             tc.tile_pool(name="tmp", bufs=3) as tmp, \
             tc.tile_pool(name="nlp", bufs=2) as nlp, \
             tc.tile_pool(name="pst", bufs=1) as pst, \
             tc.tile_pool(name="ps", bufs=1, space="PSUM") as ps, \
             tc.tile_pool(name="drm", bufs=1, space="DRAM") as drm:

            ps_ctr = [0]

            def ps_tile(shape, tag=None):
                t = ps.tile(shape, FP32, tag=tag or f"y{ps_ctr[0] % 8}")
                if tag is None:
                    ps_ctr[0] += 1
                return t

            ld_ctr = [0]

            def load(ap_in, shape, dt=BF16, pool=cst, nm=None):
                nm = nm or f"c_{ap_in.tensor.name}"
                t = pool.tile(shape, dt, tag=nm, name=nm)
                eng = nc.sync if ld_ctr[0] % 2 == 0 else nc.scalar
                ld_ctr[0] += 1
                eng.dma_start(out=t, in_=ap_in)
                return t

            # ---- constants (alternating DMA queues)
            t_xpad = load(i_xpad, [96, H + 2, W + 2])
            t_wfu = load(i_wfu, [96, 2, 9, 96])
            t_cb = load(i_cb, [96, 2], FP32)
            t_xpj = load(i_xpj, [96, 2, 44])
            t_dtw0 = load(i_dtw[0:6, :], [6, 96], nm='c_dtw0')
            t_dtw1 = load(i_dtw[6:12, :], [6, 96], nm='c_dtw1')
            t_dtws = [t_dtw0, t_dtw1]
            t_dtb = load(i_dtb, [96, 2], FP32)
            t_A = load(i_A, [96, 16], FP32)
            t_dse = load(i_dse, [96, 1], FP32)
            t_id96 = load(i_id96, [96, 96])
            t_onc = load(i_onc, [96, 1])
            t_onr = load(i_onr, [1, 96])
            t_msel = load(i_msel, [8, 2])
            t_wz = load(i_wz, [96, 96])
            t_wo = load(i_wo, [96, 96])
            t_wob = load(i_wob, [96, 96])
            t_xTh = load(i_xTh, [96, LH])
            t_eps = cst.tile([1, 1], FP32)
            nc.vector.memset(t_eps, 1e-5)

            d_xd = drm.tile([44, L], BF16)
            d_s1 = drm.tile([192, LH], BF16)
            d_r1 = drm.tile([96, LH], BF16)
            d_s2a = drm.tile([192, LH // 2], BF16)
            d_s2b = drm.tile([192, LH // 2], BF16)
            d_r2a = drm.tile([96, LH // 2], BF16)
            d_r2b = drm.tile([96, LH // 2], BF16)
            d_st_in = drm.tile([2, LH], FP32)
            d_st_out = drm.tile([8, LH], FP32)

            # ---- phase B: fused conv+in-proj on tensor + SiLU, both halves;
            # diag reorder right after each half (big segments via DMA).
            t_xs0s = [None, None]
            for h in (0, 1):
                t_xsr = tmp.tile([96, L], BF16, tag="tmp16", name=f"t_xsr{h}")
                for f in range(NF):
                    p = ps_tile([96, RPC, W])
                    for t in range(9):
                        ky, kx = t // 3, t % 3
                        rhs = t_xpad[:, f * RPC + ky:f * RPC + ky + RPC,
                                     kx:kx + W]
                        nc.tensor.matmul(p, t_wfu[:, h, t, :], rhs,
                                         start=(t == 0), stop=(t == 8))
                    p2 = p.rearrange("p r w -> p (r w)")
                    fs = slice(f * FC, (f + 1) * FC)
                    nc.scalar.activation(out=t_xsr[:, fs], in_=p2,
                                         func=AF.Silu,
                                         bias=t_cb[:, h:h + 1], scale=1.0)
                t_xs0 = big.tile([96, L], BF16, tag=f"xs0_{h}",
                                 name=f"t_xs0_{h}")
                dctr = 0
                for si, (dp, rp, ln, st) in enumerate(segs):
                    sg = t_xsr[:, rp:rp + (ln - 1) * st + 1:st] if st > 1 \
                        else t_xsr[:, rp:rp + ln]
                    if (si * 2 + h) % 4 < 3:
                        nc.vector.tensor_copy(out=t_xs0[:, dp:dp + ln], in_=sg)
                    else:
                        nc.gpsimd.tensor_copy(out=t_xs0[:, dp:dp + ln], in_=sg)
                t_xs0s[h] = t_xs0
            t_xs0 = t_xs0s[0]

            # ---- z-path: z = silu(Wz x), own half, diag order
            t_z = pst.tile([96, LH], BF16, tag="z")
            for f in range(LH // FC):
                fs = slice(f * FC, (f + 1) * FC)
                p = ps_tile([96, FC])
                nc.tensor.matmul(p, t_wz, t_xTh[:, fs], start=True, stop=True)
                nc.scalar.activation(out=t_z[:, fs], in_=p, func=AF.Silu,
                                     bias=0.0, scale=1.0)

            # ---- phase E: x_proj rows -> DRAM, dt expansion (both k)
            t_sgf = {}
            t_du = {}
            for k in (1, 0):
                t_sgf[k] = tmp.tile([96, L], BF16, tag="tmp16",
                                    name=f"t_sgf{k}")
            for f in range(NF):
                fs = slice(f * FC, (f + 1) * FC)
                for k in (1, 0):
                    p = ps_tile([22, FC])
                    for h in (0, 1):
                        nc.tensor.matmul(p, t_xpj[:, h, k * 22:(k + 1) * 22],
                                         t_xs0s[h][:, fs],
                                         start=(h == 0), stop=(h == 1))
                    t_xdc = nlp.tile([22, FC], BF16, tag="xdc", bufs=2)
                    if (f + k) % 2 == 0:
                        nc.vector.tensor_copy(out=t_xdc, in_=p)
                    else:
                        nc.scalar.copy(out=t_xdc, in_=p)
                    nc.sync.dma_start(out=d_xd[k * 22:k * 22 + 22, fs],
                                      in_=t_xdc)
                    p2 = ps_tile([96, FC])
                    nc.tensor.matmul(p2, t_dtws[k], t_xdc[0:6, :],
                                     start=True, stop=True)
                    nc.scalar.activation(out=t_sgf[k][:, fs],
                                         in_=p2, func=AF.Sigmoid,
                                         bias=t_dtb[:, k:k + 1], scale=-1.0)
            nc.scalar.activation(out=t_sgf[1], in_=t_sgf[1], func=AF.Ln,
                                 bias=0.0, scale=1.0)

            # ---- scan loop: k=1 first, then k=0; pair-RS after each
            while ps_ctr[0] % 8 != 0:
                ps_ctr[0] += 1

            for k in (1, 0):
                if k == 0:
                    nc.scalar.activation(out=t_sgf[0], in_=t_sgf[0],
                                         func=AF.Ln, bias=0.0, scale=1.0)
                t_d = big.tile([96, L], BF16, tag="du", name=f"t_du{k}")
                nc.vector.tensor_tensor(out=t_d, in0=t_sgf[k], in1=t_xs0,
                                        op=OP.mult)
                t_du[k] = t_d
                yps = [ps.tile([96, FC], FP32, tag=f"y{f}", name=f"yps{k}{f}")
                       for f in range(NF)]
                for j in range(8):
                    row_b = k * 22 + 6 + j
                    row_c = k * 22 + 14 + j
                    t_br = nlp.tile([96, L], BF16, tag="brep", bufs=2)
                    t_cr = nlp.tile([96, L], BF16, tag="crep", bufs=2)
                    nc.sync.dma_start(
                        out=t_br,
                        in_=d_xd[row_b:row_b + 1, :].to_broadcast((96, L)))
                    nc.scalar.dma_start(
                        out=t_cr,
                        in_=d_xd[row_c:row_c + 1, :].to_broadcast((96, L)))
                    t_dA = nlp.tile([96, L], BF16, tag="dA", bufs=2)
                    nc.scalar.activation(out=t_dA, in_=t_sgf[k], func=AF.Exp,
                                         bias=0.0,
                                         scale=t_A[:, k * 8 + j:k * 8 + j + 1])
                    nc.vector.tensor_tensor(out=t_br, in0=t_du[k],
                                            in1=t_br, op=OP.mult)
                    t_h = nlp.tile([96, L], BF16, tag="h", bufs=2)
                    if k == 0:
                        nc.vector.tensor_tensor_scan(
                            out=t_h, data0=t_dA, data1=t_br, initial=0.0,
                            op0=OP.mult, op1=OP.add)
                    else:
                        nc.vector.tensor_tensor_scan(
                            out=t_h[:, ::-1], data0=t_dA[:, ::-1],
                            data1=t_br[:, ::-1], initial=0.0,
                            op0=OP.mult, op1=OP.add)
                    nc.vector.tensor_tensor(out=t_h, in0=t_h, in1=t_cr,
                                            op=OP.mult)
                    for f in range(NF):
                        nc.tensor.matmul(yps[f], t_id96,
                                         t_h[:, f * FC:(f + 1) * FC],
                                         start=(j == 0), stop=(j == 7))
                # evict into the pair-RS input layout
                if k == 1:
                    forder = list(range(NF))
                    for i, f in enumerate(forder):
                        ro = (f // 4) * 96
                        cs = slice((f % 4) * FC, (f % 4) * FC + FC)
                        t_ev = nlp.tile([96, FC], BF16, tag="yev", bufs=2)
                        nc.scalar.activation(out=t_ev, in_=yps[f],
                                             func=AF.Identity, bias=0.0,
                                             scale=1.0)
                        eng = nc.sync if i % 2 == 0 else nc.scalar
                        eng.dma_start(out=d_s1[ro:ro + 96, cs], in_=t_ev)
                    nc.gpsimd.collective_compute(
                        "ReduceScatter", OP.add, replica_groups=PAIRS,
                        ins=[d_s1.opt()], outs=[d_r1.opt()])
                else:
                    # halves: a = f in (0,1,4,5) -> cols 0:1024 of own half
                    for gi, grp in enumerate(((0, 1, 4, 5), (2, 3, 6, 7))):
                        dst = d_s2a if gi == 0 else d_s2b
                        for i, f in enumerate(grp):
                            ro = (f // 4) * 96
                            cs = slice((f % 2) * FC, (f % 2) * FC + FC)
                            t_ev = nlp.tile([96, FC], BF16, tag="yev", bufs=2)
                            nc.vector.scalar_tensor_tensor(
                                out=t_ev, in0=t_xs0[:, f * FC:(f + 1) * FC],
                                scalar=t_dse, in1=yps[f], op0=OP.mult,
                                op1=OP.add)
                            eng = nc.sync if i % 2 == 0 else nc.scalar
                            eng.dma_start(out=dst[ro:ro + 96, cs], in_=t_ev)
                        nc.gpsimd.collective_compute(
                            "ReduceScatter", OP.add, replica_groups=PAIRS,
                            ins=[dst.opt()],
                            outs=[(d_r2a if gi == 0 else d_r2b).opt()])

            # ---- Q = Wo_g @ z (stats- and y-independent; fills RS#2 wait)
            t_q = pst.tile([96, LH], BF16, tag="q")
            for f in range(LH // FC):
                fs = slice(f * FC, (f + 1) * FC)
                p_qq = ps_tile([96, FC])
                nc.tensor.matmul(p_qq, t_wo, t_z[:, fs], start=True,
                                 stop=True)
                nc.vector.tensor_copy(out=t_q[:, fs], in_=p_qq)

            # ---- post: y = r1 + r2 (own-dh rows, own L-half), split halves
            t_ra = nlp.tile([96, LH], BF16, tag="rca", bufs=1)
            nc.sync.dma_start(out=t_ra, in_=d_r1)
            t_rb = nlp.tile([96, LH], BF16, tag="rcb", bufs=1)
            nc.sync.dma_start(out=t_rb[:, 0:LH // 2], in_=d_r2a)
            nc.scalar.dma_start(out=t_rb[:, LH // 2:LH], in_=d_r2b)
            t_y = t_ra
            nc.vector.tensor_tensor(out=t_y[:, 0:LH // 2],
                                    in0=t_ra[:, 0:LH // 2],
                                    in1=t_rb[:, 0:LH // 2], op=OP.add)
            nc.vector.tensor_tensor(out=t_y[:, LH // 2:LH],
                                    in0=t_ra[:, LH // 2:LH],
                                    in1=t_rb[:, LH // 2:LH], op=OP.add)

            # LN stats: own-dh partial sums / sumsq -> AllGather quad
            for f in range(LH // FC):
                fs = slice(f * FC, (f + 1) * FC)
                t_ysq = nlp.tile([96, FC], BF16, tag="ysq", bufs=2)
                nc.scalar.activation(out=t_ysq, in_=t_y[:, fs],
                                     func=AF.Square, bias=0.0, scale=1.0)
                p_s = ps_tile([1, FC])
                nc.tensor.matmul(p_s, t_onc, t_y[:, fs], start=True,
                                 stop=True)
                p_q = ps_tile([1, FC])
                nc.tensor.matmul(p_q, t_onc, t_ysq, start=True,
                                 stop=True)
                t_stc = nlp.tile([1, FC], FP32, tag="stc", bufs=2)
                nc.vector.tensor_copy(out=t_stc, in_=p_s)
                t_stq = nlp.tile([1, FC], FP32, tag="stq", bufs=2)
                nc.vector.tensor_copy(out=t_stq, in_=p_q)
                nc.sync.dma_start(out=d_st_in[0:1, fs], in_=t_stc)
                nc.scalar.dma_start(out=d_st_in[1:2, fs], in_=t_stq)
            nc.gpsimd.collective_compute(
                "AllGather", OP.bypass, replica_groups=QUADS,
                ins=[d_st_in.opt()], outs=[d_st_out.opt()])

            # During AG flight: P1 = Wo_g @ (y*z); copies land in SBUF.
            t_p1 = pst.tile([96, LH], BF16, tag="p1")
            for f in range(LH // FC):
                fs = slice(f * FC, (f + 1) * FC)
                t_yz = nlp.tile([96, FC], BF16, tag="ysq", bufs=2)
                nc.vector.tensor_tensor(out=t_yz, in0=t_y[:, fs],
                                        in1=t_z[:, fs], op=OP.mult)
                p_p1 = ps_tile([96, FC])
                nc.tensor.matmul(p_p1, t_wo, t_yz, start=True, stop=True)
                nc.scalar.copy(out=t_p1[:, fs], in_=p_p1)

            # after AG: selector-matmul stats, mu/rstd rows, rescale rows
            for f in range(LH // FC):
                fs = slice(f * FC, (f + 1) * FC)
                t_staf = nlp.tile([8, FC], FP32, tag="stalf", bufs=2)
                eng = nc.sync if f % 2 == 0 else nc.scalar
                eng.dma_start(out=t_staf, in_=d_st_out[:, fs])
                t_sta = nlp.tile([8, FC], BF16, tag="stall", bufs=2)
                nc.vector.tensor_copy(out=t_sta, in_=t_staf)
                p_m = ps_tile([1, FC])
                nc.tensor.matmul(p_m, t_msel[:, 0:1], t_sta, start=True,
                                 stop=True)
                p_m2 = ps_tile([1, FC])
                nc.tensor.matmul(p_m2, t_msel[:, 1:2], t_sta, start=True,
                                 stop=True)
                t_mub = nlp.tile([1, FC], BF16, tag="mub", bufs=2)
                nc.vector.tensor_scalar_mul(out=t_mub, in0=p_m,
                                            scalar1=1.0 / D)
                t_m2 = nlp.tile([1, FC], FP32, tag="m2", bufs=2)
                nc.vector.tensor_scalar_mul(out=t_m2, in0=p_m2,
                                            scalar1=1.0 / D)
                t_mu2 = nlp.tile([1, FC], FP32, tag="mu2", bufs=2)
                nc.vector.tensor_tensor(out=t_mu2, in0=t_mub, in1=t_mub,
                                        op=OP.mult)
                t_var = nlp.tile([1, FC], FP32, tag="var", bufs=2)
                nc.vector.tensor_tensor(out=t_var, in0=t_m2, in1=t_mu2,
                                        op=OP.subtract)
                t_vre = nlp.tile([1, FC], FP32, tag="vre", bufs=2)
                nc.vector.tensor_scalar_add(out=t_vre, in0=t_var,
                                            scalar1=1e-5)
                t_iva = nlp.tile([1, FC], FP32, tag="iva", bufs=2)
                nc.vector.reciprocal_approx_fast(out=t_iva, in_=t_vre)
                t_rsb = nlp.tile([1, FC], BF16, tag="rsb", bufs=2)
                nc.scalar.activation(out=t_rsb, in_=t_iva, func=AF.Sqrt,
                                     bias=0.0, scale=1.0)

                p_bm = ps_tile([96, FC])
                nc.tensor.matmul(p_bm, t_onr, t_mub, start=True, stop=True)
                p_br = ps_tile([96, FC])
                nc.tensor.matmul(p_br, t_onr, t_rsb, start=True, stop=True)
                t_c1 = nlp.tile([96, FC], BF16, tag="c1", bufs=2)
                nc.vector.tensor_tensor(out=t_c1, in0=t_q[:, fs], in1=p_bm,
                                        op=OP.mult)
                t_c2 = nlp.tile([96, FC], BF16, tag="c2", bufs=2)
                nc.vector.tensor_tensor(out=t_c2, in0=t_p1[:, fs], in1=t_c1,
                                        op=OP.subtract)
                t_oc = nlp.tile([96, FC], FP32, tag="oc", bufs=2)
                if has_beta:
                    t_c3 = nlp.tile([96, FC], BF16, tag="c3", bufs=2)
                    nc.vector.tensor_tensor(out=t_c3, in0=t_c2, in1=p_br,
                                            op=OP.mult)
                    p_b3 = ps_tile([96, FC])
                    nc.tensor.matmul(p_b3, t_wob, t_z[:, fs], start=True,
                                     stop=True)
                    nc.vector.tensor_tensor(out=t_oc, in0=t_c3, in1=p_b3,
                                            op=OP.add)
                else:
                    nc.vector.tensor_tensor(out=t_oc, in0=t_c2, in1=p_br,
                                            op=OP.mult)
                nc.sync.dma_start(out=o_out[:, fs], in_=t_oc)

    nc.finalize()
    return nc


def get_program(H=64, has_beta=False):
    key = (H, has_beta)
    if key not in _PROGRAMS:
        _PROGRAMS[key] = build_program(H, has_beta)
    return _PROGRAMS[key]


# ---------------------------------------------------------------- host side

def make_in_maps(inputs, H=64):
    W = H
    L = H * W
    LH = L // 2
    bf = ml_dtypes.bfloat16
    order = diag_order(H)

    x = np.asarray(inputs["x"], np.float32)
    w_in = np.asarray(inputs["w_in"], np.float32)
    conv_w = np.asarray(inputs["conv_w"], np.float32)
    conv_b = np.asarray(inputs["conv_b"], np.float32)
    x_proj_w = np.asarray(inputs["x_proj_w"], np.float32)
    dt_w = np.asarray(inputs["dt_w"], np.float32)
    dt_b = np.asarray(inputs["dt_b"], np.float32)
    A_logs = np.asarray(inputs["A_logs"], np.float32)
    Ds = np.asarray(inputs["Ds"], np.float32)
    ln_g = np.asarray(inputs["ln_g"], np.float32)
    ln_b = np.asarray(inputs["ln_b"], np.float32)
    w_out = np.asarray(inputs["w_out"], np.float32)

    A_full = np.exp(A_logs).reshape(K, D, N)
    Ds2 = Ds.reshape(K, D)
    id96 = np.eye(96, dtype=np.float32)
    onc = np.ones((96, 1), np.float32)
    onr = np.ones((1, 96), np.float32)

    in_maps = []
    for c in range(NCORES):
        b, dh, nh = c // 4, (c // 2) % 2, c % 2
        dsl = slice(dh * 96, dh * 96 + 96)
        osl = slice((1 - dh) * 96, (1 - dh) * 96 + 96)

        xpad = np.zeros((96, H + 2, W + 2), np.float32)
        xpad[:, 1:H + 1, 1:W + 1] = np.transpose(x[b], (2, 0, 1))
        xT = x[b].reshape(L, DM).T
        xTh = np.ascontiguousarray(xT[:, order[nh * LH:(nh + 1) * LH]])

        # conv fused with in-proj (tensor path), both halves
        wfu = np.zeros((96, 2, 9, 96), np.float32)
        for hh, hsl in enumerate((dsl, osl)):
            cw = conv_w[hsl, 0].reshape(96, 9)
            for t in range(9):
                wfu[:, hh, t, :] = (w_in[hsl, :] * cw[:, t:t + 1]).T

        cb2 = np.stack([conv_b[dsl], conv_b[osl]], axis=1)
        xpj = np.zeros((96, 2, 44), np.float32)
        for hh, hsl in enumerate((dsl, osl)):
            for k in range(K):
                sel = np.concatenate([
                    x_proj_w[k, 0:R, hsl],
                    -x_proj_w[k, R + nh * 8:R + nh * 8 + 8, hsl],
                    x_proj_w[k, R + N + nh * 8:R + N + nh * 8 + 8, hsl],
                ], axis=0)
                xpj[:, hh, k * 22:(k + 1) * 22] = sel.T

        dtw = np.zeros((12, 96), np.float32)
        for k in range(K):
            dtw[k * 6:(k + 1) * 6, :] = dt_w[k, dsl, :].T

        dtb = -np.stack([dt_b[0, dsl], dt_b[1, dsl]], axis=1)
        Acol = np.concatenate([A_full[0, dsl, nh * 8:nh * 8 + 8],
                               A_full[1, dsl, nh * 8:nh * 8 + 8]], axis=1)
        dse = ((Ds2[0, dsl] + Ds2[1, dsl]) / 2.0)[:, None]
        msel = np.zeros((8, 2), np.float32)
        for rr in (dh * 2 + nh, (1 - dh) * 2 + nh):
            msel[rr * 2, 0] = 1.0
            msel[rr * 2 + 1, 1] = 1.0
        wz = np.ascontiguousarray(w_in[D + dh * 96:D + dh * 96 + 96, :].T)
        wo = np.ascontiguousarray(w_out[:, dsl].T * ln_g[dsl][:, None])
        wob = np.ascontiguousarray(w_out[:, dsl].T * ln_b[dsl][:, None])

        in_maps.append({
            "xpad": xpad.reshape(96, -1).astype(bf),
            "xTh": xTh.astype(bf),
            "wfu": wfu.reshape(96, -1).astype(bf),
            "cb": cb2.astype(np.float32),
            "xpj": xpj.reshape(96, -1).astype(bf),
            "dtw": dtw.astype(bf),
            "dtb": dtb.astype(np.float32),
            "Acol": Acol.astype(np.float32),
            "dse": dse.astype(np.float32),
            "id96": id96.astype(bf),
            "onc": onc.astype(bf),
            "onr": onr.astype(bf),
            "msel": msel.astype(bf),
            "wz": wz.astype(bf),
            "wo": wo.astype(bf),
            "wob": wob.astype(bf),
        })
    return in_maps


def assemble_output(results, H=64):
    L = H * H
    LH = L // 2
    order = diag_order(H)
    out = np.zeros((B, L, DM), np.float32)
    for b in range(B):
        for nh in range(2):
            acc = (results[b * 4 + nh]["out_part"] +
                   results[b * 4 + 2 + nh]["out_part"])
            out[b, order[nh * LH:(nh + 1) * LH], :] = acc.T
    return out.reshape(B, H, H, DM)


def kernel(**inputs):
    has_beta = bool(np.any(np.asarray(inputs["ln_b"]) != 0.0))
    nc = get_program(64, has_beta)
    in_maps = make_in_maps(inputs, 64)
    res = run_bass_kernel_spmd(nc, in_maps, core_ids=list(range(NCORES)))
    return assemble_output(res.results, 64)
